# revision 1
# baseline (speedup 1.0000x reference)
"""v3 Trainium2 kernel for nn_Augmenter: two-pass gather design.

Per core (16 images):
  pass 1: 16 dma_gathers (256 idxs, elem = one 3-channel padded row, 1536B)
          -> col-major XT[p, g, jj, r], row shift via indices (zero row pads)
  batched color math + mask on VectorE (~25 big ops, broadcast APs)
  store1 -> staging[g, 320colpad, 3, 256] fp16 (2 DMAs; pads zeroed once)
  pass 2: 16 dma_gathers (col shift via indices, zero col pads)
          -> row-major OUT2[p, g, jj=2c+j2, c'']
  3 final fp16 stores; host converts to f32.

Math: y = A*x + Bp*MC + D;  A = cf*s, Bp = cf*(1-s)/3, D = dm*S/196608 + b
  cf = con+.5, s = 2*sat, b = br-.5, dm = 1-cf, S = sum over image of MC.
Mask (pre-col-shift coords): mT = 1 - rcut[r]*ccut_shifted[col].
"""

import numpy as np

import concourse.bacc as bacc
import concourse.bass as bass
import concourse.mybir as mybir
import concourse.tile as tile

F32 = mybir.dt.float32
F16 = mybir.dt.float16
I16 = mybir.dt.int16
I32 = mybir.dt.int32
OP = mybir.AluOpType
AX = mybir.AxisListType

N_CORES = 8
B_FULL = 128
N = B_FULL // N_CORES  # 16 images per core
C, H, W = 3, 256, 256
PLANE = H * W
ROWS = 320  # padded rows (32 | 256 | 32)
EL = C * W  # 768 elems per gather element (3-channel row), 1536B fp16


def build_kernel(repeat: int = 1, mode: str = "full"):
    # mode: full | nog2 (skip gather2+final stores) | gonly (gathers only)
    #       | noc (no compute: gathers + stores of raw data)
    nc = bacc.Bacc(
        "TRN2",
        target_bir_lowering=False,
        debug=False,
        enable_asserts=False,
        num_devices=N_CORES,
    )
    imgsp_t = nc.dram_tensor("imgsp", [N * ROWS, EL], F16, kind="ExternalInput")
    prm_t = nc.dram_tensor("prm", [1, 7 * N], F32, kind="ExternalInput")
    idxb_t = nc.dram_tensor("idxb", [128, 16], F32, kind="ExternalInput")
    out_t = nc.dram_tensor("out", [N * C * PLANE], F16, kind="ExternalOutput")
    imgsp = imgsp_t.ap()
    prm = prm_t.ap()
    idxb = idxb_t.ap()
    out = out_t.ap()

    with tile.TileContext(nc) as tc:
        with (
            tc.tile_pool(name="cst", bufs=1) as cpool,
            tc.tile_pool(name="big", bufs=3) as bigpool,
            tc.tile_pool(name="mid", bufs=1) as midpool,
            tc.tile_pool(name="sm", bufs=1) as smpool,
            tc.tile_pool(name="ix", bufs=1) as ixpool,
            tc.tile_pool(name="ps", bufs=2, space="PSUM") as pspool,
            tc.tile_pool(name="dr", bufs=1, space="DRAM") as drpool,
        ):
            V = nc.vector

            # ---------- setup (outside repeat loop, like baseline) ----------
            PRM = cpool.tile([128, 7 * N], F32)
            nc.sync.dma_start(PRM, prm.broadcast_to([128, 7 * N]))
            BR = PRM[:, 0 * N : 1 * N]
            SAT = PRM[:, 1 * N : 2 * N]
            CON = PRM[:, 2 * N : 3 * N]
            TX = PRM[:, 3 * N : 4 * N]
            TY = PRM[:, 4 * N : 5 * N]
            CX = PRM[:, 5 * N : 6 * N]
            CY = PRM[:, 6 * N : 7 * N]

            IDXB = cpool.tile([128, 16], F32)  # 32 + s*16 + p%16
            nc.sync.dma_start(IDXB, idxb)

            CR = cpool.tile([128, 12 * N], F32)
            cf = CR[:, 0 * N : 1 * N]
            A = CR[:, 1 * N : 2 * N]
            Bp = CR[:, 2 * N : 3 * N]
            bb = CR[:, 3 * N : 4 * N]
            dmp = CR[:, 4 * N : 5 * N]
            txg = CR[:, 5 * N : 6 * N]  # 320*g + tx - 32
            tyg = CR[:, 6 * N : 7 * N]  # 320*g + ty - 32
            lox = CR[:, 7 * N : 8 * N]
            hix = CR[:, 8 * N : 9 * N]
            s2t = CR[:, 9 * N : 10 * N]
            lorv = CR[:, 10 * N : 11 * N]
            hirv = CR[:, 11 * N : 12 * N]

            LOHI = cpool.tile([128, 3 * N], F32)
            loyv = LOHI[:, 0:N]
            hiy = LOHI[:, N : 2 * N]
            tysv = LOHI[:, 2 * N : 3 * N]

            G320_i = cpool.tile([128, N], I32)
            nc.gpsimd.iota(G320_i, pattern=[[320, N]], base=0, channel_multiplier=0)
            G320f = cpool.tile([128, N], F32)
            V.tensor_copy(G320f, G320_i)

            V.tensor_scalar(cf, CON, 1.0, 0.5, OP.mult, OP.add)
            V.tensor_scalar(s2t, SAT, 2.0, None, OP.mult)
            V.tensor_tensor(A, cf, s2t, OP.mult)
            V.tensor_tensor(Bp, cf, A, OP.subtract)  # cf(1-s)
            V.tensor_scalar(Bp, Bp, 1.0 / 3.0, None, OP.mult)
            V.tensor_scalar(bb, BR, 1.0, -0.5, OP.mult, OP.add)
            V.tensor_scalar(dmp, cf, -1.0 / 196608.0, 1.0 / 196608.0, OP.mult, OP.add)
            V.tensor_scalar(txg, TX, 1.0, -32.0, OP.mult, OP.add)
            V.tensor_tensor(txg, txg, G320f, OP.add)
            V.tensor_scalar(tyg, TY, 1.0, -32.0, OP.mult, OP.add)
            V.tensor_tensor(tyg, tyg, G320f, OP.add)
            V.tensor_scalar(lorv, TX, -1.0, 32.0, OP.mult, OP.add)
            V.tensor_scalar(hirv, TX, -1.0, 287.0, OP.mult, OP.add)
            V.tensor_scalar(lox, CX, 64.0, 0.0, OP.subtract, OP.max)
            V.tensor_scalar(hix, CX, 63.0, 255.0, OP.add, OP.min)
            V.tensor_scalar(tysv, TY, 1.0, -32.0, OP.mult, OP.add)
            V.tensor_scalar(loyv, CY, 64.0, 0.0, OP.subtract, OP.max)
            V.tensor_tensor(loyv, loyv, tysv, OP.add)
            V.tensor_scalar(hiy, CY, 63.0, 255.0, OP.add, OP.min)
            V.tensor_tensor(hiy, hiy, tysv, OP.add)

            IOTR_i = cpool.tile([128, 256], I32)
            nc.gpsimd.iota(IOTR_i, pattern=[[1, 256]], base=0, channel_multiplier=0)
            IOTR = cpool.tile([128, 256], F32)
            V.tensor_copy(IOTR, IOTR_i)
            IOP2_i = cpool.tile([128, 2], I32)
            nc.gpsimd.iota(IOP2_i, pattern=[[128, 2]], base=0, channel_multiplier=1)
            IOP2 = cpool.tile([128, 2], F32)
            V.tensor_copy(IOP2, IOP2_i)

            # ccT[p, j, g] = (loy <= j*128+p <= hiy)
            ccT = cpool.tile([128, 2, N], F32)
            e1 = cpool.tile([128, 2, N], F32)
            V.tensor_tensor(
                e1,
                IOP2.unsqueeze(2).broadcast_to([128, 2, N]),
                loyv.unsqueeze(1).broadcast_to([128, 2, N]),
                OP.is_ge,
            )
            V.tensor_tensor(
                ccT,
                IOP2.unsqueeze(2).broadcast_to([128, 2, N]),
                hiy.unsqueeze(1).broadcast_to([128, 2, N]),
                OP.is_le,
            )
            V.tensor_tensor(ccT, ccT, e1, OP.logical_and)

            ONESC = cpool.tile([128, 1], F32)
            V.memset(ONESC, 1.0)
            ONESR = cpool.tile([1, 128], F32)
            V.memset(ONESR, 1.0)

            # staging DRAM + zero col-pads once (32 small DMAs)
            STG = drpool.tile([N * ROWS, EL], F16)
            stg3 = STG.rearrange("(g q) e -> g q e", g=N)
            ZP = cpool.tile([128, 192], F16)
            V.memset(ZP, 0.0)
            for g in range(N):
                for lo in (0, 288):
                    nc.sync.dma_start(
                        stg3[g, lo : lo + 32, :]
                        .rearrange("q e -> (q e)")
                        .rearrange("(p f) -> p f", p=128),
                        ZP,
                    )

            # ---------------- steady-state pipeline ----------------
            imgsp3 = imgsp.rearrange("(g q) e -> g q e", g=N)

            def _tail(rep):
                if mode == "nog2":
                    return
                # pass-2: 8 gathers of 512 idxs (2 images each, g = 2b+gg)
                ix2f = ixpool.tile([128, 8, 2, 16], F32, tag="ixf", name="ix2f")
                V.tensor_tensor(
                    ix2f,
                    IDXB.unsqueeze(1).unsqueeze(2).broadcast_to([128, 8, 2, 16]),
                    tyg.rearrange("p (b gg) -> p b gg", b=8)
                    .unsqueeze(3)
                    .broadcast_to([128, 8, 2, 16]),
                    OP.add,
                )
                ix2 = ixpool.tile([128, 8, 2, 16], I16, tag="ixi", name="ix2")
                V.tensor_copy(ix2, ix2f)

                OUT2 = bigpool.tile([128, 8, 6, 512], F16, tag="big", name="OUT2")
                for b in range(8):
                    nc.gpsimd.dma_gather(
                        out_ap=OUT2[:, b, :, :],
                        in_ap=STG,
                        idxs_ap=ix2[:, b, :, :],
                        num_idxs=512,
                        num_idxs_reg=512,
                        elem_size=EL,
                        transpose=True,
                    )

                outv = out.rearrange(
                    "(g c j p f) -> g c j p f", g=N, c=C, j=2, p=128
                )
                for c in range(C):
                    for j2 in range(2):
                        for gg in range(2):
                            nc.sync.dma_start(
                                outv[gg : N : 2, c, j2, :, :].rearrange(
                                    "b p f -> p b f"
                                ),
                                OUT2[:, :, 2 * c + j2, 256 * gg : 256 * gg + 256],
                            )

            for rep in range(repeat):
                # unshifted load for the contrast mean (reference computes m0
                # over the ORIGINAL image, before translation)
                XTu = bigpool.tile([128, N, 2, EL], F16, tag="big", name="XTu")
                for j in range(2):
                    nc.scalar.dma_start(
                        XTu[:, :, j, :],
                        imgsp3[:, 32 + 128 * j : 160 + 128 * j, :].rearrange(
                            "g p e -> p g e"
                        ),
                    )
                s1 = smpool.tile([128, N, 2], F32, tag="s1")
                V.tensor_reduce(
                    s1.rearrange("p n j -> p (n j)"),
                    XTu.rearrange("p n j e -> p (n j) e"),
                    AX.X,
                    OP.add,
                )
                mcp = smpool.tile([128, N], F32, tag="mcp")
                V.tensor_reduce(mcp, s1, AX.X, OP.add)

                psS = pspool.tile([1, N], F32, tag="psS")
                nc.tensor.matmul(psS, lhsT=ONESC, rhs=mcp, start=True, stop=True)
                Drow = smpool.tile([1, N], F32, tag="Drow")
                V.tensor_tensor(Drow, psS, dmp[0:1, :], OP.mult)
                V.tensor_tensor(Drow, Drow, bb[0:1, :], OP.add)
                psD = pspool.tile([128, N], F32, tag="psD")
                nc.tensor.matmul(psD, lhsT=ONESR, rhs=Drow, start=True, stop=True)

                ix1f = ixpool.tile([128, N, 16], F32, tag="ixf", name="ix1f")
                V.tensor_tensor(
                    ix1f,
                    IDXB.unsqueeze(1).broadcast_to([128, N, 16]),
                    txg.unsqueeze(2).broadcast_to([128, N, 16]),
                    OP.add,
                )
                ix1 = ixpool.tile([128, N, 16], I16, tag="ixi", name="ix1")
                V.tensor_copy(ix1, ix1f)

                XT = bigpool.tile([128, N, 6, 256], F16, tag="big", name="XT")
                for g in range(N):
                    nc.gpsimd.dma_gather(
                        out_ap=XT[:, g, :, :],
                        in_ap=imgsp,
                        idxs_ap=ix1[:, g, :],
                        num_idxs=256,
                        num_idxs_reg=256,
                        elem_size=EL,
                        transpose=True,
                    )

                if mode == "gonly":
                    continue
                if mode == "noc":
                    OM = XT
                    for j in range(2):
                        nc.scalar.dma_start(
                            stg3[:, 32 + 128 * j : 160 + 128 * j, :].rearrange(
                                "g p e -> p g e"
                            ),
                            OM[:, :, 3 * j : 3 * j + 3, :],
                        )
                    _tail(rep)
                    continue
                x0 = XT[:, :, 0:6:3, :]
                x1 = XT[:, :, 1:6:3, :]
                x2 = XT[:, :, 2:6:3, :]
                # tmp2 accumulates: MC = x0+x1+x2, then Bp*MC + D in place
                tmp2 = midpool.tile([128, N, 2, 256], F16, tag="tmp2")
                V.tensor_tensor(tmp2, x0, x1, OP.add)
                V.tensor_tensor(tmp2, tmp2, x2, OP.add)

                V.tensor_tensor(
                    tmp2,
                    tmp2,
                    Bp.unsqueeze(2).unsqueeze(3).broadcast_to([128, N, 2, 256]),
                    OP.mult,
                )
                V.tensor_tensor(
                    tmp2,
                    tmp2,
                    psD.unsqueeze(2).unsqueeze(3).broadcast_to([128, N, 2, 256]),
                    OP.add,
                )

                rcB = midpool.tile([128, N, 256], F16, tag="rcB")
                e2 = midpool.tile([128, N, 256], F16, tag="mT", name="e2")
                V.tensor_tensor(
                    e2,
                    IOTR.unsqueeze(1).broadcast_to([128, N, 256]),
                    lox.unsqueeze(2).broadcast_to([128, N, 256]),
                    OP.is_ge,
                )
                V.tensor_tensor(
                    rcB,
                    IOTR.unsqueeze(1).broadcast_to([128, N, 256]),
                    hix.unsqueeze(2).broadcast_to([128, N, 256]),
                    OP.is_le,
                )
                V.tensor_tensor(rcB, rcB, e2, OP.logical_and)
                # row validity rv[g,r] = (lorv <= r <= hirv); u = rcut*rv
                rvB = midpool.tile([128, N, 256], F16, tag="rvB")
                V.tensor_tensor(
                    e2,
                    IOTR.unsqueeze(1).broadcast_to([128, N, 256]),
                    lorv.unsqueeze(2).broadcast_to([128, N, 256]),
                    OP.is_ge,
                )
                V.tensor_tensor(
                    rvB,
                    IOTR.unsqueeze(1).broadcast_to([128, N, 256]),
                    hirv.unsqueeze(2).broadcast_to([128, N, 256]),
                    OP.is_le,
                )
                V.tensor_tensor(rvB, rvB, e2, OP.logical_and)
                V.tensor_tensor(rcB, rcB, rvB, OP.mult)  # u = rcut*rv
                # mT = rv - ccT*u
                mT = midpool.tile([128, N, 2, 256], F16, tag="mT")
                V.tensor_tensor(
                    mT,
                    ccT.transpose([0, 2, 1])
                    .unsqueeze(3)
                    .broadcast_to([128, N, 2, 256]),
                    rcB.unsqueeze(2).broadcast_to([128, N, 2, 256]),
                    OP.mult,
                )
                V.tensor_tensor(
                    mT,
                    rvB.unsqueeze(2).broadcast_to([128, N, 2, 256]),
                    mT,
                    OP.subtract,
                )

                # fold mask: tmp2 *= mT ; mT *= A ; o_c = x_c*mT + tmp2
                V.tensor_tensor(tmp2, tmp2, mT, OP.mult)
                AB = A.unsqueeze(2).unsqueeze(3).broadcast_to([128, N, 2, 256])
                V.tensor_tensor(mT, mT, AB, OP.mult)
                OM = bigpool.tile([128, N, 6, 256], F16, tag="big", name="OM")
                for c in range(C):
                    xc = XT[:, :, c : 6 : 3, :]
                    oc = OM[:, :, c : 6 : 3, :]
                    V.tensor_tensor(oc, xc, mT, OP.mult)
                    V.tensor_tensor(oc, oc, tmp2, OP.add)

                # store1: OM[p, g, 3j+c, r] -> staging[g, 32+j*128+p, (c r)]
                for j in range(2):
                    nc.scalar.dma_start(
                        stg3[:, 32 + 128 * j : 160 + 128 * j, :].rearrange(
                            "g p e -> p g e"
                        ),
                        OM[:, :, 3 * j : 3 * j + 3, :],
                    )

                _tail(rep)

    nc.compile()
    return nc


# ---------------------------------------------------------------------------
# Host wrapper: full-input kernel() over 8 NeuronCores
# ---------------------------------------------------------------------------

from concourse.bass_utils import run_bass_kernel_spmd

_CACHE = {}

_P = np.arange(128)[:, None]
_S = np.arange(16)[None, :]
_IDXB = (32 + _S * 16 + (_P % 16)).astype(np.float32)


def _get_compiled(repeat):
    if repeat not in _CACHE:
        _CACHE[repeat] = build_kernel(repeat)
    return _CACHE[repeat]


def _pack_core(imgs, br, sat, con, tx, ty, cx, cy):
    buf = np.zeros((N, ROWS, 2, C, 128), np.float16)
    b2 = imgs.transpose(0, 2, 1, 3).reshape(N, H, C, 2, 128)
    buf[:, 32 : 32 + H] = b2.transpose(0, 1, 3, 2, 4)
    prm = np.concatenate(
        [
            br.reshape(N), sat.reshape(N), con.reshape(N),
            tx.reshape(N), ty.reshape(N), cx.reshape(N), cy.reshape(N),
        ]
    ).astype(np.float32)[None, :]
    return {
        "imgsp": buf.reshape(N * ROWS, EL),
        "prm": prm,
        "idxb": _IDXB,
    }


def kernel(imgs, br, sat, con, tx, ty, cx, cy, _repeat=1):
    imgs = np.asarray(imgs, np.float32).astype(np.float16)
    br = np.asarray(br, np.float32)
    sat = np.asarray(sat, np.float32)
    con = np.asarray(con, np.float32)
    tx = np.asarray(tx, np.int32).astype(np.float32)
    ty = np.asarray(ty, np.int32).astype(np.float32)
    cx = np.asarray(cx, np.int32).astype(np.float32)
    cy = np.asarray(cy, np.int32).astype(np.float32)

    nc = _get_compiled(_repeat)
    in_maps = []
    for k in range(N_CORES):
        sl = slice(k * N, (k + 1) * N)
        in_maps.append(
            _pack_core(
                imgs[sl], br[sl], sat[sl], con[sl], tx[sl], ty[sl], cx[sl], cy[sl]
            )
        )
    res = run_bass_kernel_spmd(nc, in_maps, core_ids=list(range(N_CORES)))
    out = np.empty((N_CORES * N, C, H, W), np.float32)
    for k in range(N_CORES):
        out[k * N : (k + 1) * N] = (
            np.asarray(res.results[k]["out"]).astype(np.float32).reshape(N, C, H, W)
        )
    return out

